# revision 3
# baseline (speedup 1.0000x reference)
"""Distributed Trainium2 kernel for nn_Block_44951127720380.

Strategy: 8 NeuronCores, data-parallel. Core c handles batch b = c//4 and
query slice [qs, qs+512) with qs = (c%4)*512. Each core receives its
batch's full token sequence (needed for K/V + the shift-by-one reference
point) plus replicated weights, computes LN1 -> logmap -> QKV -> causal
attention (its 512 queries vs all 2048 keys) -> expmap -> aproj ->
residual -> LN2 -> FC -> gelu -> mproj -> residual for its slice, and
returns a [512, 1024] output shard. No cross-core collectives are needed;
the host reassembles the [2, 2048, 1024] output.
"""

import numpy as np

B, T, C, H = 2, 2048, 1024, 16
D = C // H
CURV = 1.0
EPS = 1e-5
N_CORES = 8
TP = 4          # query-slices per batch
QS = T // TP    # 512 queries per core

_compiled = None


def _build():
    import jax
    import jax.numpy as jnp

    def layer_norm(x, w, b):
        mu = jnp.mean(x, axis=-1, keepdims=True)
        var = jnp.var(x, axis=-1, keepdims=True)
        return (x - mu) * jax.lax.rsqrt(var + EPS) * w + b

    def mobius_addition(x, y, c):
        xn2 = jnp.sum(x * x, axis=-1, keepdims=True)
        yn2 = jnp.sum(y * y, axis=-1, keepdims=True)
        ip = jnp.sum(x * y, axis=-1, keepdims=True)
        num = (1 + 2 * c * ip + c * yn2) * x + (1 - c * xn2) * y
        den = 1 + 2 * c * ip + (c ** 2) * xn2 * yn2
        return num / den

    def scaling_factor(x, c):
        xn2 = jnp.sum(x * x, axis=-1, keepdims=True)
        return 2.0 / (1.0 + c * xn2)

    def atanh(z):
        # mhlo.atanh does not lower on the neuron backend
        return 0.5 * (jnp.log1p(z) - jnp.log1p(-z))

    def expmap(x, v, c):
        lam = scaling_factor(x, c)
        vn = jnp.sqrt(jnp.sum(v * v, axis=-1, keepdims=True))
        second = (1.0 / c ** 0.5) * jnp.tanh(jnp.sqrt(c * lam * vn ** 2 / 2.0)) * v / vn
        return mobius_addition(x, second, c)

    def logmap(x, u, c):
        lam = scaling_factor(x, c)
        m = mobius_addition(-x, u, c)
        mn = jnp.sqrt(jnp.sum(m * m, axis=-1, keepdims=True))
        cf = 2.0 / (lam * c ** 0.5)
        arg = jnp.clip(jnp.sqrt(c * mn), -0.999, 0.999)
        return cf * atanh(arg) * m / mn

    def core_fn2(qs_idx, xb, x_my, mask, ln1_w, ln1_b, W_attn, b_attn,
                 W_aproj, b_aproj, ln2_w, ln2_b, W_fc, b_fc, W_mproj, b_mproj):
        c = CURV
        xn = layer_norm(xb, ln1_w, ln1_b)
        ref = jnp.pad(xn[:-1, :], ((1, 0), (0, 0)))
        xt = logmap(ref, xn, c)
        qkv = xt @ W_attn + b_attn
        q, k, v = jnp.split(qkv, 3, axis=-1)
        q_my = jax.lax.dynamic_slice_in_dim(q, qs_idx[0], QS, 0)   # [QS, C]
        qh = q_my.reshape(QS, H, D).transpose(1, 0, 2)             # [H, QS, D]
        kh = k.reshape(T, H, D).transpose(1, 0, 2)
        vh = v.reshape(T, H, D).transpose(1, 0, 2)
        scores = jnp.einsum('hqd,hkd->hqk', qh, kh) / np.float32(np.sqrt(D))
        scores = scores + mask[None, :, :]
        att = jax.nn.softmax(scores, axis=-1)
        y = jnp.einsum('hqk,hkd->hqd', att, vh)
        y = y.transpose(1, 0, 2).reshape(QS, C)
        ref_my = jax.lax.dynamic_slice_in_dim(ref, qs_idx[0], QS, 0)
        y = expmap(ref_my, y, c)
        y = y @ W_aproj + b_aproj
        x2 = x_my + y
        h = layer_norm(x2, ln2_w, ln2_b)
        h = jax.nn.gelu(h @ W_fc + b_fc, approximate=False)
        h = h @ W_mproj + b_mproj
        return x2 + h

    fn = jax.pmap(
        core_fn2,
        in_axes=(0, 0, 0, 0) + (None,) * 12,
        devices=jax.devices()[:N_CORES],
    )
    return fn


def kernel(x, ln1_w, ln1_b, W_attn, b_attn, W_aproj, b_aproj,
           ln2_w, ln2_b, W_fc, b_fc, W_mproj, b_mproj):
    global _compiled
    import jax
    if _compiled is None:
        _compiled = _build()

    x = np.asarray(x, dtype=np.float32)
    # Per-core shards
    xb = np.stack([x[c // TP] for c in range(N_CORES)])            # [8, T, C]
    x_my = np.stack([x[c // TP, (c % TP) * QS:(c % TP + 1) * QS]
                     for c in range(N_CORES)])                     # [8, QS, C]
    qs_idx = np.array([[(c % TP) * QS] for c in range(N_CORES)],
                      dtype=np.int32)                              # [8, 1]
    # additive causal mask per core: query row i (global (c%TP)*QS + i)
    # may attend keys <= global index
    key_idx = np.arange(T)
    masks = []
    for c in range(N_CORES):
        qglob = (c % TP) * QS + np.arange(QS)
        m = np.where(key_idx[None, :] <= qglob[:, None], 0.0, -1e30)
        masks.append(m.astype(np.float32))
    masks = np.stack(masks)                                        # [8, QS, T]

    out_shards = _compiled(
        qs_idx, xb, x_my, masks,
        np.asarray(ln1_w, np.float32), np.asarray(ln1_b, np.float32),
        np.asarray(W_attn, np.float32), np.asarray(b_attn, np.float32),
        np.asarray(W_aproj, np.float32), np.asarray(b_aproj, np.float32),
        np.asarray(ln2_w, np.float32), np.asarray(ln2_b, np.float32),
        np.asarray(W_fc, np.float32), np.asarray(b_fc, np.float32),
        np.asarray(W_mproj, np.float32), np.asarray(b_mproj, np.float32),
    )
    out_shards = np.asarray(out_shards)                            # [8, QS, C]
    out = np.empty((B, T, C), dtype=np.float32)
    for c in range(N_CORES):
        out[c // TP, (c % TP) * QS:(c % TP + 1) * QS] = out_shards[c]
    return out
